# revision 6
# baseline (speedup 1.0000x reference)
"""Rowwise cosine-similarity kernel for Trainium2 (8 NeuronCores, SPMD).

Computes out[b, n] = sum_d(an * bn) where an, bn are L2-normalized rows of
a, b [16, 4096, 256] -> out [16, 4096].

Sharding: 65536 rows split across 8 cores (8192 rows/core). Per core the
row slice is viewed as [128 partitions, 64 subtiles, 256], i.e. row
p*64 + t lives at partition p, subtile t. Everything is contiguous DMA.

Per 256-wide subtile three fused multiply+accum-reduce ops:
  P  = sum(a*b)   Sa = sum(a*a)   Sb = sum(b*b)
spread across DVE (scalar_tensor_tensor), ACT (Square + accum_out) and
Pool/GpSimd (scalar_tensor_tensor) so no single engine is the bottleneck;
the kernel is then limited by the 16-SDMA-engine input stream (~40 us).
All chunk buffers are SBUF-resident and every input DMA is issued up
front, so the DMA stream is never throttled by compute.
Finalize (batched): out = P * sqrt(1/(Sa*Sb)).
"""

import sys

if "/opt/trn_rl_repo" not in sys.path:
    sys.path.insert(0, "/opt/trn_rl_repo")

import numpy as np
import orjson

import concourse.bass as bass
import concourse.mybir as mybir
import concourse.tile as tile
from concourse import bass2jax, bass_utils

# ---------------------------------------------------------------------------
# Environment patches.
#
# 1. No cloud share in this sandbox: upload_artifacts would fail.
# 2. The walrus build here accepts at most ONE semaphore wait per
#    instruction; the Tile scheduler freely attaches several.  Post-process
#    the BIR before compiling: move surplus waits onto single-wait Drain
#    carrier instructions inserted just before the original instruction on
#    the same engine queue.
# ---------------------------------------------------------------------------

bass_utils.upload_artifacts = lambda tmpdir: ""

_MAX_WAITS = 1


def _split_bir_waits(bir_json: bytes) -> bytes:
    d = orjson.loads(bir_json)
    ctr = 0
    for fn in d.get("functions", []):
        for blk in fn.get("blocks", []):
            insts = blk.get("instructions")
            if not insts:
                continue
            out = []
            for inst in insts:
                si = inst.get("sync_info")
                waits = (si or {}).get("on_wait") or []
                if len(waits) > _MAX_WAITS:
                    surplus = waits[:-_MAX_WAITS]
                    si["on_wait"] = waits[-_MAX_WAITS:]
                    for i in range(0, len(surplus), _MAX_WAITS):
                        out.append(
                            {
                                "name": f"WSPL-{ctr}",
                                "opcode": "Drain",
                                "engine": inst["engine"],
                                "ins": [],
                                "outs": [],
                                "is_reset_sema": False,
                                "debug": inst.get("debug", 0),
                                "sync_info": {
                                    "on_wait": surplus[i : i + _MAX_WAITS],
                                    "on_update": [],
                                },
                            }
                        )
                        ctr += 1
                out.append(inst)
            blk["instructions"] = out
    return orjson.dumps(d)


def _exempt_sp_from_entry_barrier(d: dict) -> None:
    """Let the SP (DMA-issuing) engine skip the kernel-entry barrier.

    The entry barrier only protects the const-AP memsets, which SP never
    reads; removing SP's blocking wait lets input DMAs start ~5 us earlier.
    The leader's release count is reduced so both sems still return to 0.
    """
    blk = d["functions"][0]["blocks"][0]
    insts = blk["instructions"]
    if not any(i.get("opcode") == "Memset" for i in insts):
        return
    sp_idx = None
    pool_add = None
    for i, inst in enumerate(insts):
        if inst.get("opcode") != "EventSemaphore":
            continue
        si = inst.get("sync_info") or {}
        ow = si.get("on_wait") or []
        ou = si.get("on_update") or []
        if not ou:
            continue
        u0 = ou[0]
        if "release" not in str(u0.get("ant_name", "")):
            continue
        if inst.get("engine") == "SP" and u0.get("update_mode") == "sem-dec":
            sp_idx = i
        if (
            inst.get("engine") == "Pool"
            and not ow
            and u0.get("update_mode") == "sem-add-imm"
        ):
            pool_add = inst
    if sp_idx is not None and pool_add is not None:
        uv = pool_add["sync_info"]["on_update"][0]
        if uv["update_value"] >= 2:
            del insts[sp_idx]
            uv["update_value"] -= 1


_orig_compile_bir_kernel = bass_utils.compile_bir_kernel


def _patched_compile_bir_kernel(bir_json, tmpdir, neff_name="file.neff"):
    if isinstance(bir_json, str):
        bir_json = bir_json.encode()
    d = orjson.loads(_split_bir_waits(bir_json))
    _exempt_sp_from_entry_barrier(d)
    bir_json = orjson.dumps(d)
    return _orig_compile_bir_kernel(bir_json, tmpdir, neff_name=neff_name)


bass_utils.compile_bir_kernel = _patched_compile_bir_kernel
bass2jax.compile_bir_kernel = _patched_compile_bir_kernel

from concourse.vector_clock import ScopedClock  # noqa: E402


def _lean_drain_and_barrier(self, tick_clock, wait_clock):
    """Tile kernel tail without the trailing all-engine barrier.

    After the first barrier every engine is done with real work; gpsimd
    clears the semaphores and each engine halts independently (NRT waits
    for all engines anyway), so the second barrier only adds latency.
    """
    drain_inst = self.nc.sync.drain()
    wait_clock.add_sem_waits(
        drain_inst.ins, ScopedClock({None: tick_clock.global_clock})
    )
    self.nc.all_engine_barrier()
    popped = self.nc._tile_sem_poison_stack.pop()
    assert popped is self._sem_poison
    self.nc.clear_and_free_semaphores(list(self.sems.allocated().values()))


tile.TileContext._drain_and_barrier = _lean_drain_and_barrier

# ---------------------------------------------------------------------------
# Problem constants (hardcoded; kernel.py must be self-contained).
# ---------------------------------------------------------------------------

N_CORES = 8
B, N, D = 16, 4096, 256
ROWS = B * N                     # 65536
ROWS_PER_CORE = ROWS // N_CORES  # 8192
P = 128                          # SBUF partitions
T = ROWS_PER_CORE // P           # 64 subtiles per core
COLS = T * D                     # 16384 dram cols per partition

# Chunk sizes in subtiles (128 KB per subtile per tensor, both tensors
# SBUF-resident for every chunk). Small first chunk starts compute early;
# small last chunk keeps the drain tail short.
CHUNK_PLAN = (2, 6, 8, 8, 8, 8, 8, 8, 6, 2)
assert sum(CHUNK_PLAN) == T

# Engine load split for the 192 fused reduce ops (64 P + 128 squares).
# Per-op busy: DVE stt ~346 ns, ACT square+accum ~585 ns. Pool is unusable
# (its SBUF port is shared with DVE: any Pool op serializes 2-port DVE ops).
N_ACT_SQ = 71   # squares on ACT (of 2*T = 128 square slots); rest on DVE

# Finalize column ranges, keyed by the chunk index after which they run
# (one chunk of lag so finalize never stalls an engine's program order).
FIN_PLAN = {4: (0, 24), 6: (24, 40), 8: (40, 56), 9: (56, 64)}


def _bres(i: int, n: int, m: int) -> bool:
    """Evenly spread n picks over m slots (Bresenham)."""
    return (i * n) // m != ((i + 1) * n) // m


_CACHE: dict = {}


def _build_bass():
    f32 = mybir.dt.float32
    alu = mybir.AluOpType
    act = mybir.ActivationFunctionType

    nc = bass.Bass(
        "TRN2",
        debug=False,
        num_devices=N_CORES,
        enable_asserts=False,
        enable_partition_id=False,
    )
    a_d = nc.dram_tensor("a", (P, COLS), f32, kind="ExternalInput").ap()
    b_d = nc.dram_tensor("b", (P, COLS), f32, kind="ExternalInput").ap()
    o_d = nc.dram_tensor("out", (P, T), f32, kind="ExternalOutput").ap()

    with tile.TileContext(nc) as tc:
        with (
            tc.tile_pool(name="stats", bufs=1) as stats_pool,
            tc.tile_pool(name="chunks", bufs=1) as chunk_pool,
            tc.tile_pool(name="dscr", bufs=3) as dve_scr,
            tc.tile_pool(name="ascr", bufs=3) as act_scr,
            tc.tile_pool(name="fin", bufs=1) as fin_pool,
        ):
            p_t = stats_pool.tile([P, T], f32, tag="p")
            sa_t = stats_pool.tile([P, T], f32, tag="sa")
            sb_t = stats_pool.tile([P, T], f32, tag="sb")
            denom = fin_pool.tile([P, T], f32, tag="denom")
            rec = fin_pool.tile([P, T], f32, tag="rec")
            rsq = fin_pool.tile([P, T], f32, tag="rsq")
            out_t = fin_pool.tile([P, T], f32, tag="out")

            # All chunk buffers resident; every input DMA issued up front so
            # the SDMA engines stream back-to-back, unthrottled by compute.
            a_chs, b_chs = [], []
            c0 = 0
            for ch, ct in enumerate(CHUNK_PLAN):
                a_ch = chunk_pool.tile([P, ct * D], f32, tag=f"a{ch}")
                b_ch = chunk_pool.tile([P, ct * D], f32, tag=f"b{ch}")
                nc.sync.dma_start(a_ch[:], a_d[:, c0 * D : (c0 + ct) * D])
                nc.sync.dma_start(b_ch[:], b_d[:, c0 * D : (c0 + ct) * D])
                a_chs.append(a_ch)
                b_chs.append(b_ch)
                c0 += ct

            # Engine picker: ACT takes N_ACT_SQ of the 2*T square slots
            # (Bresenham-spread); everything else runs on DVE.
            state = {"sq": 0}

            def pick_square() -> str:
                s = state["sq"]
                state["sq"] += 1
                return "act" if _bres(s, N_ACT_SQ, 2 * T) else "dve"

            def emit_stt(sub0, sub1, dst, t: int):
                scr = dve_scr.tile([P, D], f32, tag="dve", name="dvescr")
                nc.vector.scalar_tensor_tensor(
                    out=scr[:],
                    in0=sub0,
                    scalar=0.0,
                    in1=sub1,
                    op0=alu.add,
                    op1=alu.mult,
                    accum_out=dst[:, t : t + 1],
                )

            def emit_square(eng_name: str, sub, dst, t: int):
                if eng_name == "act":
                    scr = act_scr.tile([P, D], f32, tag="act", name="actscr")
                    nc.scalar.activation(
                        scr[:], sub, act.Square, accum_out=dst[:, t : t + 1]
                    )
                else:
                    emit_stt(sub, sub, dst, t)

            def finalize(lo: int, hi: int):
                nc.vector.tensor_mul(
                    denom[:, lo:hi], sa_t[:, lo:hi], sb_t[:, lo:hi]
                )
                nc.vector.reciprocal(rec[:, lo:hi], denom[:, lo:hi])
                nc.scalar.activation(rsq[:, lo:hi], rec[:, lo:hi], act.Sqrt)
                nc.vector.tensor_mul(
                    out_t[:, lo:hi], p_t[:, lo:hi], rsq[:, lo:hi]
                )
                nc.sync.dma_start(o_d[:, lo:hi], out_t[:, lo:hi])

            tbase = 0
            for ch, ct in enumerate(CHUNK_PLAN):
                for s in range(ct):
                    t = tbase + s
                    asub = a_chs[ch][:, s * D : (s + 1) * D]
                    bsub = b_chs[ch][:, s * D : (s + 1) * D]
                    emit_stt(asub, bsub, p_t, t)
                    emit_square(pick_square(), asub, sa_t, t)
                    emit_square(pick_square(), bsub, sb_t, t)
                tbase += ct
                if ch in FIN_PLAN:
                    finalize(*FIN_PLAN[ch])

    return nc


def _get_nc():
    if "nc" not in _CACHE:
        _CACHE["nc"] = _build_bass()
    return _CACHE["nc"]


def kernel(a: np.ndarray, b: np.ndarray) -> np.ndarray:
    a = np.ascontiguousarray(np.asarray(a, dtype=np.float32)).reshape(ROWS, D)
    b = np.ascontiguousarray(np.asarray(b, dtype=np.float32)).reshape(ROWS, D)

    in_maps = []
    for c in range(N_CORES):
        sl = slice(c * ROWS_PER_CORE, (c + 1) * ROWS_PER_CORE)
        in_maps.append(
            {"a": a[sl].reshape(P, COLS), "b": b[sl].reshape(P, COLS)}
        )

    nc = _get_nc()
    res = bass_utils.run_bass_kernel_spmd(nc, in_maps, core_ids=list(range(N_CORES)))
    out = np.concatenate(
        [res.results[c]["out"].reshape(ROWS_PER_CORE) for c in range(N_CORES)]
    )
    return out.reshape(B, N)
